# revision 3
# baseline (speedup 1.0000x reference)
"""Trainium2 Bass kernel for the 4-branch "Memory multimode" attention module.

Reference computation (per batch element b):
    q  = q_key[b].reshape(1024, 128)        (row-major reinterpret)
    pq = p_q_key[b].reshape(1024, 128)
    k  = m_key[b].reshape(128, 2048)
    pk = p_m_key[b].reshape(128, 2048)
    mval = m_val[b].reshape(512, 2048).T    # [2048, 512]
    out  = (sm(q@k) + sm(pq@pk) + sm(pq@k) + sm(q@pk)) @ mval
    where sm() is softmax over the QUERY dim (axis 0 of each [1024, 2048] score
    matrix).  Final output channel-concats q_val.

All four branches share the value matrix, so the four softmax matrices are
summed BEFORE the value matmul - one [1024,2048]@[2048,512] matmul instead of
four.

Implementation (one NeuronCore per batch element, 8 cores, data-parallel).
The ACT engine's exp sweep (64 x [128,1024] tiles, ~0.83ns/elem, dtype-
independent) is the hard floor (~71us); everything else is sized to hide
under it:
  * Transposed score layout S^T = [key_pos(l) x query(i)]: softmax reduction
    runs along the free dim; S^T tiles come straight off the PE with
    lhsT = keys l-tile (natural layout) and rhs = Q^T (host pre-transposed).
  * Scores in plain f16 (single matmul per tile): ~6e-3 absolute score error
    -> ~2e-3 relative output error, well inside the 2e-2 gate, and 3x fewer
    PE instructions than the previous bf16 hi/lo 3-term scheme.
  * No max-subtraction needed: |scores| <= ~75, exp stays in fp32/bf16 range.
    ScalarE exp emits bf16 E tiles plus the softmax denominators via
    accum_out (fused row-sum).
  * The 1/D scaling + 4-branch sum runs on the DVE as a tensor_scalar /
    scalar_tensor_tensor chain with all-16-bit streams (bf16 E in, f16 tmp/
    A^T out) - qualifies for the DVE 4x_2p mode (~0.26ns/elem).
  * Value matmul in f16 (1 cyc/row).  4 of the 8 output row-blocks
    accumulate in dedicated PSUM banks during phase 1, software-pipelined
    one tile behind the scores so PE never stalls the ACT stream; the other
    4 run as a tail re-using the score PSUM banks.
  * Output staging copies run on the otherwise-idle Pool (gpsimd) engine.
  * q_val never touches the device: concatenated on the host.
"""

import numpy as np

import concourse.bass as bass
import concourse.mybir as mybir
import concourse.tile as tile
from concourse.bass_utils import run_bass_kernel_spmd
from concourse.vector_clock import ScopedClock

# The walrus build in this image supports only ONE sync-wait command per
# instruction (CTRL_NO_STRUCT / S3_LW_STRUCT encodings); this concourse's Tile
# scheduler freely attaches several.  Two fixes: (1) split the kernel-tail
# drain's waits over several drains, (2) a post-scheduling pass that moves
# overflow waits onto NoOps inserted before the over-subscribed instruction.
_MAX_WAITS = 1


def _split_drain_and_barrier(self, tick_clock, wait_clock):
    nc = self.nc
    drain_inst = nc.sync.drain()
    wait_clock.add_sem_waits(
        drain_inst.ins, ScopedClock({None: tick_clock.global_clock})
    )
    mi = drain_inst.ins
    waits = list(mi.sync_info.on_wait)
    if len(waits) > _MAX_WAITS:
        del mi.sync_info.on_wait[_MAX_WAITS:]
        rest = waits[_MAX_WAITS:]
        for i in range(0, len(rest), _MAX_WAITS):
            extra = nc.sync.drain()
            if extra.ins.sync_info is None:
                extra.ins.sync_info = mybir.SyncInfo(on_wait=[], on_update=[])
            extra.ins.sync_info.on_wait.extend(rest[i : i + _MAX_WAITS])

    nc.all_engine_barrier()
    assert self.sems is not None
    popped = nc._tile_sem_poison_stack.pop()
    assert popped is self._sem_poison
    nc.clear_and_free_semaphores(list(self.sems.allocated().values()))
    nc.all_engine_barrier()


tile.TileContext._drain_and_barrier = _split_drain_and_barrier


def _split_sync_waits(nc, cap: int = _MAX_WAITS):
    for f in nc.m.functions:
        for blk in f.blocks:
            out = []
            changed = False
            for inst in blk.instructions:
                si = inst.sync_info
                if si is not None and len(si.on_wait) > cap:
                    waits = list(si.on_wait)
                    rest, keep = waits[:-cap], waits[-cap:]
                    for i in range(0, len(rest), cap):
                        noop = mybir.InstNoOp(
                            name=nc.get_next_instruction_name(), ins=[], outs=[]
                        )
                        noop.engine = inst.engine
                        noop.sync_info = mybir.SyncInfo(
                            on_wait=rest[i : i + cap], on_update=[]
                        )
                        nc.register_instruction(noop)
                        out.append(noop)
                    inst.sync_info = mybir.SyncInfo(
                        on_wait=keep, on_update=list(si.on_update)
                    )
                    changed = True
                out.append(inst)
            if changed:
                blk.instructions = out
    return nc


B, H, W = 8, 32, 32
HW = H * W          # 1024 queries
KD = 128            # key dim
VD = 512            # val dim
L = 2 * HW          # 2048 key positions per key matrix
NT = L // 128       # 16 l-tiles
NCORES = 8

F32 = mybir.dt.float32
F16 = mybir.dt.float16
BF16 = mybir.dt.bfloat16

_nc_cache = {}


def build_nc(n_overlap: int = 4):
    """n_overlap: output-row PSUM accumulations interleaved into phase 1
    (each holds one PSUM bank for the whole phase; score tiles use 4)."""
    nc = bass.Bass("TRN2", target_bir_lowering=False, debug=False)

    def din(name, shape, dt):
        return nc.dram_tensor(name, shape, dt, kind="ExternalInput").ap()

    mk, pmk = din("mk", [KD, L], F16), din("pmk", [KD, L], F16)
    qt, pqt = din("qt", [KD, HW], F16), din("pqt", [KD, HW], F16)
    mvt = din("mvt", [L, VD], F16)
    out = nc.dram_tensor("out", [HW, VD], F32, kind="ExternalOutput").ap()

    EXP = mybir.ActivationFunctionType.Exp
    MUL = mybir.AluOpType.mult
    ADD = mybir.AluOpType.add
    NO = HW // 128  # 8 output row-tiles

    with tile.TileContext(nc) as tc:
        with (
            tc.tile_pool(name="keys", bufs=1) as keys_pool,
            tc.tile_pool(name="qts", bufs=1) as qt_pool,
            tc.tile_pool(name="mv", bufs=1) as mv_pool,
            tc.tile_pool(name="ework", bufs=3) as e_pool,
            tc.tile_pool(name="atiles", bufs=1) as a_pool,
            tc.tile_pool(name="dwork", bufs=3) as d_pool,
            tc.tile_pool(name="ostage", bufs=2) as out_pool,
            tc.tile_pool(name="psum_s", bufs=2, space="PSUM") as psum_s,
            tc.tile_pool(name="psum_o", bufs=1, space="PSUM") as psum_o,
        ):
            # ---- input loads, ordered so the first score matmuls start early:
            # queries first, then the first l-half of each key tensor, then the
            # rest; value tiles last (not needed until the first A^T exists).
            keys = keys_pool.tile([128, 2 * L], F16, tag="keys")
            qts = qt_pool.tile([128, 2 * HW], F16, tag="qts")
            for c in range(2):
                nc.sync.dma_start(qts[:, c * 512 : (c + 1) * 512],
                                  qt[:, c * 512 : (c + 1) * 512])
            nc.sync.dma_start(qts[:, HW:], pqt)
            for half in range(2):
                sl_d = slice(half * L // 2, (half + 1) * L // 2)
                for y, src in enumerate((mk, pmk)):
                    nc.sync.dma_start(
                        keys[:, y * L + half * L // 2 :
                             y * L + (half + 1) * L // 2],
                        src[:, sl_d])

            mv_tiles = []
            for t in range(NT):
                mvtile = mv_pool.tile([128, VD], F16, tag=f"mv{t}")
                nc.sync.dma_start(mvtile[:], mvt[t * 128 : (t + 1) * 128, :])
                mv_tiles.append(mvtile)

            # phase-1-resident output accumulators (one PSUM bank each)
            o_acc = [
                psum_o.tile([128, VD], F32, tag=f"O{i}", name=f"o_acc{i}")
                for i in range(n_overlap)
            ]

            a_tiles = []

            def emit_value(t):
                # overlapped value-matmul accumulation, pipelined one tile
                # behind the scores so the chain->value dependency never
                # stalls the next tile's score matmuls (PE issues in order).
                for i in range(n_overlap):
                    nc.tensor.matmul(
                        o_acc[i][:],
                        a_tiles[t][:, i * 128 : (i + 1) * 128],
                        mv_tiles[t][:],
                        start=(t == 0),
                        stop=(t == NT - 1),
                    )

            # ---- phase 1 ---------------------------------------------------
            for t in range(NT):
                dtile = d_pool.tile([128, 4], F32, tag="D")
                e_tiles = []
                for y in range(2):
                    e_t = e_pool.tile([128, 2 * HW], BF16, tag=f"E{y}")
                    kslice = slice(y * L + t * 128, y * L + (t + 1) * 128)
                    for xh in range(2):
                        s_ps = psum_s.tile([128, HW], F32, tag="S")
                        for c in range(2):
                            qslice = slice(xh * HW + c * 512,
                                           xh * HW + (c + 1) * 512)
                            nc.tensor.matmul(
                                s_ps[:, c * 512 : (c + 1) * 512],
                                keys[:, kslice], qts[:, qslice],
                                start=True, stop=True)
                        # E^T = exp(S^T) in bf16; accum_out = row sum = denom
                        nc.scalar.activation(
                            e_t[:, xh * HW : (xh + 1) * HW],
                            s_ps[:],
                            EXP,
                            accum_out=dtile[:, 2 * y + xh : 2 * y + xh + 1],
                        )
                    e_tiles.append(e_t)

                invd = d_pool.tile([128, 4], F32, tag="invD")
                nc.vector.reciprocal(invd[:], dtile[:])

                # A^T[t] = sum_{y,xh} invD * E-half; all-16-bit streams keep
                # the DVE in 4x mode.
                a_sb = a_pool.tile([128, HW], F16, tag=f"A{t}")
                tmp0 = d_pool.tile([128, HW], F16, tag="atmp", name=f"t0_{t}")
                nc.vector.tensor_scalar_mul(
                    tmp0[:], e_tiles[0][:, 0:HW], invd[:, 0:1])
                tmp1 = d_pool.tile([128, HW], F16, tag="atmp", name=f"t1_{t}")
                nc.vector.scalar_tensor_tensor(
                    tmp1[:], e_tiles[0][:, HW:], invd[:, 1:2], tmp0[:],
                    MUL, ADD)
                tmp2 = d_pool.tile([128, HW], F16, tag="atmp", name=f"t2_{t}")
                nc.vector.scalar_tensor_tensor(
                    tmp2[:], e_tiles[1][:, 0:HW], invd[:, 2:3], tmp1[:],
                    MUL, ADD)
                nc.vector.scalar_tensor_tensor(
                    a_sb[:], e_tiles[1][:, HW:], invd[:, 3:4], tmp2[:],
                    MUL, ADD)
                a_tiles.append(a_sb)

                if t >= 1:
                    emit_value(t - 1)
            emit_value(NT - 1)

            # ---- phase 2: drain overlapped rows (copies/DMAs overlap the
            # tail matmuls), then the remaining row-blocks ------------------
            def stage_out(i, o_ps):
                o_sb = out_pool.tile([128, VD], F32, tag="osb",
                                     name=f"osb{i}")
                # GPSIMD can't read PSUM; DVE has slack and ACT is the
                # bottleneck, so stage on the DVE
                nc.vector.tensor_copy(o_sb[:], o_ps[:])
                nc.sync.dma_start(out[i * 128 : (i + 1) * 128, :], o_sb[:])

            for i in range(n_overlap):
                stage_out(i, o_acc[i])
            for i in range(n_overlap, NO):
                o_ps = psum_s.tile([128, VD], F32, tag="S", name=f"o_tail{i}")
                for t in range(NT):
                    nc.tensor.matmul(
                        o_ps[:],
                        a_tiles[t][:, i * 128 : (i + 1) * 128],
                        mv_tiles[t][:],
                        start=(t == 0),
                        stop=(t == NT - 1),
                    )
                stage_out(i, o_ps)

    _split_sync_waits(nc)
    return nc


def make_in_maps(m_key, m_val, q_key, p_m_key, p_q_key):
    in_maps = []
    for b in range(B):
        m = {
            "mk": np.ascontiguousarray(
                m_key[b].reshape(KD, L).astype(np.float16)),
            "pmk": np.ascontiguousarray(
                p_m_key[b].reshape(KD, L).astype(np.float16)),
            "qt": np.ascontiguousarray(
                q_key[b].reshape(HW, KD).T.astype(np.float16)),
            "pqt": np.ascontiguousarray(
                p_q_key[b].reshape(HW, KD).T.astype(np.float16)),
            "mvt": np.ascontiguousarray(
                m_val[b].reshape(VD, L).T.astype(np.float16)),
        }
        in_maps.append(m)
    return in_maps


def run(inputs, trace: bool = False, n_overlap: int = 4):
    """Run on the 8 NeuronCores; returns (full_output, BassKernelResults)."""
    inputs = {k: np.asarray(v, dtype=np.float32) for k, v in inputs.items()}
    key = (n_overlap,)
    if key not in _nc_cache:
        _nc_cache[key] = build_nc(n_overlap)
    nc = _nc_cache[key]
    in_maps = make_in_maps(
        inputs["m_key"], inputs["m_val"], inputs["q_key"],
        inputs["p_m_key"], inputs["p_q_key"],
    )
    res = run_bass_kernel_spmd(nc, in_maps, list(range(NCORES)), trace=trace)
    q_val = inputs["q_val"]
    outs = []
    for b in range(B):
        mat = np.asarray(res.results[b]["out"])      # [1024, 512] row-major
        attn = mat.reshape(VD, H, W)                 # reinterpret, no transpose
        outs.append(np.concatenate([attn, q_val[b]], axis=0))
    return np.stack(outs), res


def kernel(**inputs) -> np.ndarray:
    out, _ = run(inputs, trace=False)
    return out
